# revision 43
# baseline (speedup 1.0000x reference)
"""Trainium2 Bass kernel for nn_MultiHeadAttention_30846455119878.

8-core strategy:
  - Attention is head-sharded: core m owns heads {2m, 2m+1}. Each core projects
    q/k/v for its 2 heads over all B*T tokens, then causal softmax attention.
  - Heads -> tokens reshard via one AllToAll per batch; each core then runs the
    full output projection (contraction over all 1024 features) for its 1/8
    token slice.
  - Host side: x passed pre-transposed as x^T [C, B*T] bf16; all matmuls bf16.

v2 layout/scheduling (HW-profiled against the 478us v1):
  - Score MMs for the 2 heads are row-tiled (64-contraction) and issued
    back-to-back into one [128,2,512] PSUM tile -> they stream concurrently.
  - One exp + one mask-mul per key block covers both heads ([128,2,512-c0]).
  - Attention output goes through the collective UNNORMALIZED with the
    per-(head,token) reciprocals as 2 extra rows (cc buffers are [8,130,TS]);
    normalization happens post-collective with a stride-0 broadcast DMA of the
    reciprocals + one DVE multiply. This kills the reciprocal-broadcast
    matmuls and the pre-collective normalize multiplies of v1.
  - Output projection for batch b is emitted after collective b+1 (1-batch
    lag) so it fills the old batch-boundary PE gaps and nothing but batch 3's
    collective + outproj remains on the tail.
  - tcb loop runs descending (dense 16-block chunk first) to keep the PE HAM
    clock warm after collective-entry gaps.
  - rcv reads are split per source core j (8 DMAs) instead of one big
    rearrange gather; const DMAs ordered so wq/wk/wv+xt precede mask/wo/bo.

Perf log (HW, 8 axon trn2 cores):
  v1 (baseline): 470-478us, rel 3.8e-3. PE busy 378us, exp 185us, last
     AllToAll + cold outproj tail ~58us, head gap 25.6us, batch0 HAM-cold.
  v2: see test runs.
"""

import sys

if "/opt/trn_rl_repo" not in sys.path:
    sys.path.insert(0, "/opt/trn_rl_repo")

import numpy as np
import ml_dtypes

import concourse.bass as bass
import concourse.tile as tile
from concourse import bacc, mybir
from concourse.bass_utils import run_bass_kernel_spmd
from concourse.tile_rust import add_dep_helper

BF16 = ml_dtypes.bfloat16
USE_FP8_QK = False
FP8 = ml_dtypes.float8_e4m3fn  # values stay << 240, where TRN fp8e4 matches

# Full problem dims
B_FULL, T_FULL, C_FULL, H_FULL, D_HEAD = 4, 2048, 1024, 16, 64
N_CORES = 8
HPC = H_FULL // N_CORES  # heads per core = 2
F = HPC * D_HEAD         # per-core attention feature rows = 128
TCH = 512                # query-chunk (free dim of score matmuls)
D = D_HEAD


def build_nc(B=B_FULL, T=T_FULL, C=C_FULL):
    """Build the SPMD Bass graph (same graph on all 8 cores)."""
    dt = mybir.dt
    CK = C // 128        # contraction chunks for projections
    NTC = T // TCH       # query chunks per sequence
    NSB = T // 128       # key blocks per sequence
    SBB = TCH // 128     # key blocks overlapping one query chunk diagonal = 4
    TS = T // N_CORES    # token shard per (batch, core) = 256
    CO = H_FULL * D_HEAD  # output feature dim (Wo cols) = 1024
    TT = 128
    FE = F + HPC         # cc rows: 128 att features + 2 reciprocal rows
    # Wq/Wk are prescaled x16 on the host (fp8 subnormal-floor dodge), so the
    # scores arrive x256; fold the 1/256 into the exp scale.
    scale = float(1.0 / np.sqrt(C)) / 256.0

    CK8 = C // 256       # DoubleRow contraction chunks (256 each) for q/k
    nc = bacc.Bacc()
    xt_d = nc.declare_dram_parameter("xt", [128, CK, B * T], dt.bfloat16, isOutput=False)
    xt8_d = nc.declare_dram_parameter("xt8", [128, CK8, 2, B * T], dt.float8e4, isOutput=False)
    wq_d = nc.declare_dram_parameter("wq", [128, CK8, 2, F], dt.float8e4, isOutput=False)
    wk_d = nc.declare_dram_parameter("wk", [128, CK8, 2, F], dt.float8e4, isOutput=False)
    wv_d = nc.declare_dram_parameter("wv", [128, CK, F], dt.bfloat16, isOutput=False)
    wo_d = nc.declare_dram_parameter("wo", [128, N_CORES, CO], dt.bfloat16, isOutput=False)
    bo_d = nc.declare_dram_parameter("bo", [1, CO], dt.bfloat16, isOutput=False)
    mask_d = nc.declare_dram_parameter("mask", [128, SBB, HPC, TCH], dt.bfloat16, isOutput=False)
    out_d = nc.declare_dram_parameter("out", [B, TS, CO], dt.float32, isOutput=True)

    cc_in = [nc.dram_tensor(f"cc_in{b}", [N_CORES, FE, TS], dt.bfloat16) for b in range(B)]
    cc_out = [nc.dram_tensor(f"cc_out{b}", [N_CORES, FE, TS], dt.bfloat16) for b in range(B)]
    rg = [list(range(N_CORES))]

    with tile.TileContext(nc) as tc:
        from contextlib import ExitStack

        with ExitStack() as ctx:
            wpool = ctx.enter_context(tc.tile_pool(name="w", bufs=1))
            xpool = ctx.enter_context(tc.tile_pool(name="xt", bufs=4))
            x8pool = ctx.enter_context(tc.tile_pool(name="xt8", bufs=4))
            qkpool = ctx.enter_context(tc.tile_pool(name="qk", bufs=2))
            v1pool = ctx.enter_context(tc.tile_pool(name="v1", bufs=2))
            epool = ctx.enter_context(tc.tile_pool(name="exp", bufs=6))
            aupool = ctx.enter_context(tc.tile_pool(name="attu", bufs=2))
            denpool = ctx.enter_context(tc.tile_pool(name="den", bufs=2))
            recpool = ctx.enter_context(tc.tile_pool(name="rec", bufs=4))
            rcvpool = ctx.enter_context(tc.tile_pool(name="rcv", bufs=2))
            outpool = ctx.enter_context(tc.tile_pool(name="osb", bufs=2))
            psS = ctx.enter_context(tc.tile_pool(name="psS", bufs=3, space="PSUM"))
            psAtt = ctx.enter_context(tc.tile_pool(name="psAtt", bufs=1, space="PSUM"))

            # resident constants; emission order controls DMA start order:
            # q/k/v weights first (phase A needs them immediately), mask after
            # phase A of b0 is emitted, wo/bo only after cc(0).
            wq_sb = wpool.tile([128, CK8, 2, F], dt.float8e4, tag="wq")
            wk_sb = wpool.tile([128, CK8, 2, F], dt.float8e4, tag="wk")
            wv_sb = wpool.tile([128, CK, F], dt.bfloat16, tag="wv")
            wo_sb = wpool.tile([128, N_CORES, CO], dt.bfloat16, tag="wo")
            bo_sb = wpool.tile([1, CO], dt.bfloat16, tag="bo")
            mask_sb = wpool.tile([128, SBB, HPC, TCH], dt.bfloat16, tag="mask")
            ones_sb = wpool.tile([1, 128], dt.bfloat16, tag="ones")
            nc.vector.memset(ones_sb, 1.0)

            cc_insts = []
            xt_tiles = {}

            def emit_xt_load(b, tcb):
                g0 = b * T + tcb * TCH
                xt_sb = xpool.tile([128, CK, TCH], dt.bfloat16, tag="xt",
                                   name=f"xt_{b}_{tcb}")
                nc.sync.dma_start(out=xt_sb, in_=xt_d[:, :, g0:g0 + TCH])
                x8_sb = x8pool.tile([128, CK8, 2, TCH], dt.float8e4, tag="xt8",
                                    name=f"xt8_{b}_{tcb}")
                nc.scalar.dma_start(out=x8_sb, in_=xt8_d[:, :, :, g0:g0 + TCH])
                xt_tiles[(b, tcb)] = (xt_sb, x8_sb)

            def emit_phase_a(b):
                qT = qkpool.tile([F, T], dt.bfloat16, tag="qT", name=f"qT_{b}")
                kT = qkpool.tile([F, T], dt.bfloat16, tag="kT", name=f"kT_{b}")
                v1 = v1pool.tile([128, NSB, HPC, 80], dt.bfloat16, tag="v1", name=f"v1_{b}")
                nc.vector.memset(v1[:, :, :, D:D + 1], 1.0)
                for tcb in range(NTC):
                    xt_sb, x8_sb = xt_tiles.pop((b, tcb))
                    for w_sb, dstT in ((wq_sb, qT), (wk_sb, kT)):
                        ps = psS.tile([128, HPC, TCH], dt.float32, tag="sps")
                        for o in range(CK8):
                            nc.tensor.matmul(
                                ps[:, 0, :], lhsT=w_sb[:, o, :, :],
                                rhs=x8_sb[:, o, :, :],
                                start=(o == 0), stop=(o == CK8 - 1),
                                perf_mode=mybir.MatmulPerfMode.DoubleRow,
                            )
                        nc.vector.tensor_copy(
                            out=dstT[:, tcb * TCH:(tcb + 1) * TCH], in_=ps[:, 0, :]
                        )
                    # v directly in [s, d] layout: v[s, f] = sum_c x[s, c] Wv[c, f]
                    for ssub in range(SBB):
                        vps_full = psS.tile([128, HPC, TCH], dt.float32, tag="sps",
                                            name=f"vps_{b}_{tcb}_{ssub}")
                        vps = vps_full[:, 0, 0:F]
                        for o in range(CK):
                            nc.tensor.matmul(
                                vps,
                                lhsT=xt_sb[:, o, ssub * 128:(ssub + 1) * 128],
                                rhs=wv_sb[:, o, :],
                                start=(o == 0), stop=(o == CK - 1),
                            )
                        st = tcb * SBB + ssub
                        for h in range(HPC):
                            nc.vector.tensor_copy(
                                out=v1[:, st, h, 0:D], in_=vps[:, h * D:(h + 1) * D]
                            )
                return qT, kT, v1

            def emit_phase_b(b, qT, kT, v1):
                """Causal attention; returns list of stg DMA insts feeding cc_in[b]."""
                att_un = aupool.tile([D, HPC, NTC, TCH], dt.bfloat16, tag="attu")
                den_b = denpool.tile([D + 1, HPC * NTC * TCH], dt.float32, tag="den")
                stg_insts = []
                for tcb in range(NTC - 1, -1, -1):  # descending: dense chunk first
                    if b + 1 < B:
                        # prefetch one next-batch x chunk per tcb iteration so
                        # the 4MB doesn't monopolize the sync DMA queue
                        emit_xt_load(b + 1, NTC - 1 - tcb)
                    att_ps = psAtt.tile([D + 1, HPC, TCH], dt.float32, tag="att",
                                        name=f"attps_{b}_{tcb}")
                    nsb = SBB * (tcb + 1)
                    for sb in range(nsb):
                        j0 = sb - SBB * tcb
                        # cols t < j0*128 are fully causal-masked -> skip
                        c0 = j0 * 128 if j0 > 0 else 0
                        s_ps = psS.tile([128, HPC, TCH], dt.float32, tag="sps",
                                        name=f"sps_{b}_{tcb}_{sb}")
                        # paired row-tiled score MMs -> stream concurrently
                        for h in range(HPC):
                            nc.tensor.matmul(
                                s_ps[:, h, c0:TCH],
                                lhsT=kT[h * D:(h + 1) * D, sb * 128:(sb + 1) * 128],
                                rhs=qT[h * D:(h + 1) * D, tcb * TCH + c0:(tcb + 1) * TCH],
                                start=True, stop=True,
                                tile_position=(h * D, 0),
                            )
                        et = epool.tile([128, HPC, TCH], dt.bfloat16, tag="exp")
                        nc.scalar.activation(
                            out=et[:, :, c0:TCH], in_=s_ps[:, :, c0:TCH],
                            func=mybir.ActivationFunctionType.Exp, scale=scale,
                        )
                        if j0 >= 0:
                            nc.vector.tensor_mul(
                                et[:, :, c0:TCH], et[:, :, c0:TCH],
                                mask_sb[:, j0, :, c0:TCH],
                            )
                        # NOTE: do NOT split attV into concurrent 64-key row
                        # tiles accumulating the same bank — concurrent row
                        # tiles on one PSUM bank are a fatal HW collision
                        # (crashed on HW; tiling-doc gotcha 1).
                        for h in range(HPC):
                            nc.tensor.matmul(
                                att_ps[:, h, c0:TCH],
                                lhsT=v1[:, sb, h, 0:D + 1], rhs=et[:, h, c0:TCH],
                                start=(sb == 0), stop=(sb == nsb - 1),
                            )
                    # evacuate this tcb: unnormalized att + denominators
                    for h in range(HPC):
                        slot = tcb * HPC + h
                        nc.vector.tensor_copy(
                            out=att_un[:, h, tcb, :], in_=att_ps[0:D, h, :]
                        )
                        nc.vector.tensor_copy(
                            out=den_b[D:D + 1, slot * TCH:(slot + 1) * TCH],
                            in_=att_ps[D:D + 1, h, :],
                        )
                    # per-tcb reciprocal chain (off critical path except tcb=0)
                    den_t = recpool.tile([128, HPC * TCH // 128], dt.float32, tag="dent",
                                         name=f"dent_{b}_{tcb}")
                    nc.scalar.dma_start(
                        out=den_t,
                        in_=den_b[D:D + 1, tcb * HPC * TCH:(tcb + 1) * HPC * TCH],
                    )
                    rec_t = recpool.tile([128, HPC * TCH // 128], dt.bfloat16, tag="rect",
                                         name=f"rect_{b}_{tcb}")
                    with nc.allow_low_precision(reason="bf16 softmax denom recip ok at rel 2e-2"):
                        nc.vector.reciprocal(out=rec_t, in_=den_t)
                    rec_a = recpool.tile([1, HPC * TCH], dt.bfloat16, tag="reca",
                                         name=f"reca_{b}_{tcb}")
                    nc.scalar.dma_start(out=rec_a, in_=rec_t)
                    # stage this tcb into cc_in[b]: att rows + reciprocal rows
                    for jj in range(2):
                        j = tcb * 2 + jj
                        t0 = jj * TS
                        for h in range(HPC):
                            stg_insts.append(nc.scalar.dma_start(
                                out=cc_in[b][j, h * D:(h + 1) * D, :],
                                in_=att_un[:, h, tcb, t0:t0 + TS],
                            ).ins)
                            stg_insts.append(nc.scalar.dma_start(
                                out=cc_in[b][j, F + h:F + h + 1, :],
                                in_=rec_a[0:1, h * TCH + t0:h * TCH + t0 + TS],
                            ).ins)
                return stg_insts

            c_tiles = {}

            def emit_phase_c_dmas(b, tail=False):
                """rcv/fac reads of cc_out[b] on the gpsimd (SWDGE) queue.

                The HWDGE queues must not host these: their wait-on-cc at the
                queue head blocks every later DMA (HW-measured 33us pipeline
                freeze). The gpsimd queue only runs the collectives, so FIFO
                order there (after cc(b)) is exactly when they should fire.
                """
                rcv = rcvpool.tile([128, N_CORES, TS], dt.bfloat16, tag="rcv")
                fac = rcvpool.tile([128, N_CORES, TS], dt.bfloat16, tag="fac")
                for h in range(HPC):
                    fd = nc.gpsimd.dma_start(
                        out=fac[h * D:(h + 1) * D, :, :],
                        in_=cc_out[b][:, F + h, :].partition_broadcast(D),
                    )
                    add_dep_helper(fd.ins, cc_insts[b], sync=True, reason="cc_out RAW")
                for j in range(0, N_CORES, 2):
                    rd = nc.gpsimd.dma_start(
                        out=rcv[:, j:j + 2, :],
                        in_=cc_out[b][j:j + 2, 0:F, :].rearrange("j p t -> p j t"),
                    )
                    add_dep_helper(rd.ins, cc_insts[b], sync=True, reason="cc_out RAW")
                    if tail:
                        # last batch: normalize per j-pair on DVE (queue is
                        # empty by now) so outproj can consume rcv slices as
                        # they land instead of waiting for one big multiply
                        nc.vector.tensor_mul(
                            rcv[:, j:j + 2, :], rcv[:, j:j + 2, :], fac[:, j:j + 2, :]
                        )
                if not tail:
                    # normalize on the gpsimd engine: keeps the multiply off
                    # the DVE queue, where its wait head-of-line blocked
                    # phase B of the following batch
                    nc.gpsimd.tensor_mul(rcv, rcv, fac)
                c_tiles[b] = (rcv, fac)

            def emit_phase_c(b):
                """Output projection for this core's token shard of batch b."""
                rcv, fac = c_tiles.pop(b)
                for tt in range(TS // TT):
                    for c2 in range(CO // 512):
                        ps = psS.tile([128, HPC, TCH], dt.float32, tag="sps",
                                      name=f"ops_{b}_{tt}_{c2}")
                        for j in range(N_CORES):
                            nc.tensor.matmul(
                                ps[0:TT, 0, :],
                                lhsT=rcv[:, j, tt * TT:(tt + 1) * TT],
                                rhs=wo_sb[:, j, c2 * 512:(c2 + 1) * 512],
                                start=(j == 0), stop=False,
                            )
                        nc.tensor.matmul(
                            ps[0:TT, 0, :],
                            lhsT=ones_sb[0:1, 0:TT],
                            rhs=bo_sb[0:1, c2 * 512:(c2 + 1) * 512],
                            start=False, stop=True,
                        )
                        osb = outpool.tile([TT, 512], dt.float32, tag="osb")
                        nc.vector.tensor_copy(out=osb, in_=ps[0:TT, 0, :])
                        nc.scalar.dma_start(
                            out=out_d[b, tt * TT:(tt + 1) * TT, c2 * 512:(c2 + 1) * 512],
                            in_=osb,
                        )

            # head: interleave wq and xt8(0,0) per contraction chunk so the
            # first q matmul starts after ~100KB of DMA instead of ~1.3MB
            xt00 = xpool.tile([128, CK, TCH], dt.bfloat16, tag="xt", name="xt_0_0")
            x800 = x8pool.tile([128, CK8, 2, TCH], dt.float8e4, tag="xt8", name="xt8_0_0")
            for o in range(CK8):
                nc.sync.dma_start(out=wq_sb[:, o, :, :], in_=wq_d[:, o, :, :])
                nc.sync.dma_start(out=x800[:, o, :, :], in_=xt8_d[:, o, :, 0:TCH])
            nc.sync.dma_start(out=wk_sb, in_=wk_d[:, :, :, :])
            nc.sync.dma_start(out=wv_sb, in_=wv_d[:, :, :])
            nc.sync.dma_start(out=xt00, in_=xt_d[:, :, 0:TCH])
            xt_tiles[(0, 0)] = (xt00, x800)
            for tcb in range(1, NTC):
                emit_xt_load(0, tcb)
            for b in range(B):
                qT, kT, v1 = emit_phase_a(b)
                if b == 0:
                    # mask needed only at the first diagonal block of phase B
                    nc.sync.dma_start(out=mask_sb, in_=mask_d[:, :, :, :])
                stg_insts = emit_phase_b(b, qT, kT, v1)
                if b >= 1:
                    # rcv/fac triggers sit on the gpsimd queue right behind
                    # cc(b-1); they fire as soon as it completes
                    emit_phase_c_dmas(b - 1)
                cc = nc.gpsimd.collective_compute(
                    "AllToAll", mybir.AluOpType.bypass, replica_groups=rg,
                    ins=[cc_in[b].ap().opt()], outs=[cc_out[b].ap().opt()],
                )
                for s in stg_insts:
                    add_dep_helper(cc.ins, s, sync=True, reason="cc_in RAW")
                cc_insts.append(cc.ins)
                if b == 0:
                    # wo/bo needed first by phase C(0), emitted after cc(1)
                    nc.sync.dma_start(out=wo_sb, in_=wo_d[:, :, :])
                    nc.sync.dma_start(out=bo_sb, in_=bo_d[:, :])
                if b >= 1:
                    emit_phase_c(b - 1)
            emit_phase_c_dmas(B - 1, tail=True)
            emit_phase_c(B - 1)

    nc.finalize()
    return nc


def prep_inputs(x, Wq, Wk, Wv, Wo, bo):
    """Host-side shard/layout prep. Returns in_maps for the 8 cores."""
    B, T, C = x.shape
    CK = C // 128
    SBB = TCH // 128

    x = np.asarray(x, dtype=np.float32)
    xTf = x.reshape(B * T, C).T  # [C, B*T] fp32
    xt = np.ascontiguousarray(xTf.astype(BF16).reshape(CK, 128, B * T).transpose(1, 0, 2))
    CK8 = C // 256
    xt8 = np.ascontiguousarray(
        xTf.astype(FP8).reshape(CK8, 2, 128, B * T).transpose(2, 0, 1, 3)
    )

    CO = Wo.shape[1]
    wo_h = np.ascontiguousarray(
        np.asarray(Wo, np.float32).astype(BF16).reshape(N_CORES, 128, CO).transpose(1, 0, 2)
    )
    bo_h = np.asarray(bo, np.float32).astype(BF16).reshape(1, CO)

    p = np.arange(128)[:, None, None, None]
    j = np.arange(SBB)[None, :, None, None]
    t = np.arange(TCH)[None, None, None, :]
    mask_h = np.broadcast_to((t >= p + j * 128), (128, SBB, HPC, TCH)).astype(BF16)
    mask_h = np.ascontiguousarray(mask_h)

    in_maps = []
    for m in range(N_CORES):
        maps = {"xt": xt, "xt8": xt8, "wo": wo_h, "bo": bo_h, "mask": mask_h}
        for name, W in (("wq", Wq), ("wk", Wk)):
            # x16 prescale lifts the 0.02-sigma weights off the fp8 subnormal
            # floor; the kernel folds 1/256 back into the exp scale
            Ws = np.concatenate(
                [np.asarray(W[HPC * m + i], np.float32) for i in range(HPC)], axis=1
            ) * 16.0  # [C, F]
            maps[name] = np.ascontiguousarray(
                Ws.astype(FP8).reshape(CK8, 2, 128, F).transpose(2, 0, 1, 3)
            )
        Ws = np.concatenate(
            [np.asarray(Wv[HPC * m + i], np.float32) for i in range(HPC)], axis=1
        )
        maps["wv"] = np.ascontiguousarray(
            Ws.astype(BF16).reshape(CK, 128, F).transpose(1, 0, 2)
        )
        in_maps.append(maps)
    return in_maps


_NC_CACHE = {}


def _get_nc(B, T, C):
    key = (B, T, C)
    if key not in _NC_CACHE:
        _NC_CACHE[key] = build_nc(B, T, C)
    return _NC_CACHE[key]


def kernel(x, Wq, Wk, Wv, Wo, bo, _trace=False):
    x = np.asarray(x)
    B, T, C = x.shape
    nc = _get_nc(B, T, C)
    in_maps = prep_inputs(x, Wq, Wk, Wv, Wo, bo)
    res = run_bass_kernel_spmd(
        nc, in_maps, core_ids=list(range(N_CORES)), trace=_trace
    )
    TS = T // N_CORES
    CO = np.asarray(Wo).shape[1]
    out = np.empty((B, T, CO), dtype=np.float32)
    for m in range(N_CORES):
        out[:, m * TS:(m + 1) * TS, :] = res.results[m]["out"]
    if _trace:
        kernel.last_result = res
    return out
